# revision 33
# baseline (speedup 1.0000x reference)
"""Trainium2 Bass kernel for the MixtureLowRankRNN problem (v2).

Math background
---------------
Reference recurrence (per batch row, h0 = 0):
    h_t   = (1-a) h_{t-1} + a*scale*(m @ n.T tanh(h_{t-1})) + a*(x_t @ I.T)
    out_t = pinv([m, I]) @ h_t
Every update stays in colspan([m, I]), so h_t = m@(a*scale*A_t) + I@(a*C_t)
exactly, with
    A_t = (1-a) A_{t-1} + n.T tanh(h_{t-1})      (device: nonlinear scan)
    C_t = (1-a) C_{t-1} + x_t                    (host: x-only EMA)
and out_t = [a*scale*A_t ; a*C_t].  The scan uses the host-precomputed drive
    P_t = (1-a) P_{t-1} + a*(x_t @ I.T),  h_t = (a*scale*m)^T-exp(A_{t-1}) + P_{t-1}.

Sharding: the dynamics CONTRACT (~0.947/step: tanh is mostly in its linear
region, Jacobian ~ 0.9 I), so time segments warm-started W steps early from
A=0 converge to the true trajectory (numpy-verified; the full bf16 pipeline
at N_T=16, W=64 measures 7.4e-3 global rel err vs the 2e-2 gate).  N_T time
segments x N_G batch groups = 8 cores x K interleaved chains/core; segment 0
needs no warmup (its history IS exact), so it outputs all L steps while
segments 1.. output L-W: L = (T + (N_T-1) W) / N_T = 124 device steps.

Per-chain step (full unroll; global slot s = t*K + k on each core):
  PE : P-inject matmul (identity bf16 @ pbuf slot) -> opens ph(k) bank,
       sets has_written; 8 expansion mms (wm bf16 x A bf16) accumulate on top.
  ACT: tanh ph(k) -> th (bf16)
  PE : 8 reduction mms (wn bf16 x th bf16) -> pu(k) bank
  DVE: a_copy STT  z[(t+1)K+k] = 0.9*z[tK+k] + pu(k)   (state + history)
The reduction/a-copy/next P-inject of a chain are emitted D = K//2 slots
after its expansion (software pipelining), so the in-order engines never
block across chains; the steady cycle sits at the per-chain latency floor
(~1377ns: mm ack 173x2 + ACT 292+185 + DVE 142+125 + sem hops).  PSUM:
K x (ph + pu) = 2K banks (= 8 at K=4; one accumulation group per bank since
start=True clears a whole bank's has_written bits).  The identity rides as a
128-col prefix of pbuf so it needs no separate serialized DMA; pbuf preloads
in small-then-large chunks overlapped with compute, and the A history is
DMA'd out in chunks as it becomes final.
"""

import numpy as np
import ml_dtypes

H = 1024
R = 4
IN = 16
ALPHA = 0.1
SCALE = 500.0 / 1024.0
B = 32
T = 1024
N_CORES = 8
C = H // 128

# ---- configuration ---------------------------------------------------------
BL = 16                   # batch rows per chain
N_G = B // BL             # batch groups (2)
N_T = 16                  # time segments
K = (N_T * N_G) // N_CORES  # chains per core (4)
W = 64                    # warmup steps per segment (segment 0 needs none)
# Uneven segments: segment 0 outputs all L steps (its warmup is exact anyway);
# segments 1.. warm up W steps and output L-W.  L + (N_T-1)(L-W) = T.
L = (T + (N_T - 1) * W) // N_T   # device steps per chain (124)
assert L + (N_T - 1) * (L - W) == T
# per-segment (global output start, warmup steps)
SEG_START = [0] + [L + (j - 1) * (L - W) for j in range(1, N_T)]
SEG_WARM = [0] + [W] * (N_T - 1)
F = C * BL                # ph free width (128)
D = max(K // 2, 0)        # software-pipeline lag (slots)
Q_P = True                # pbuf (P drive) in bf16
Q_R = True                # reduction path (wn, th) in bf16
Q_S = True                # state path (z, wm, aout) in bf16
CHUNK_STEPS = 12          # pbuf DMA chunk granularity (steps)
N_OUT_DMA = 8             # streamed output DMA chunks

_CACHE = {}
LAST_RESULTS = None


def _patch_drain(tile_mod):
    """This walrus build allows only ONE semaphore wait per instruction, but
    TileContext's tail drain waits on every active proc.  Split it: one
    single-wait NOP per proc, then a waitless drain."""
    if getattr(tile_mod.TileContext, "_drain_patched", False):
        return
    from concourse.vector_clock import ScopedClock, VectorClock

    def _drain_and_barrier(self, tick_clock, wait_clock):
        gc = tick_clock.global_clock
        n = len(gc)
        for proc in range(n):
            t = gc[proc]
            if t > 0:
                nop_inst = self.nc.sync.nop()
                vec = [0] * n
                vec[proc] = t
                wait_clock.add_sem_waits(
                    nop_inst.ins, ScopedClock({None: VectorClock(vec)})
                )
        self.nc.sync.drain()
        self.nc.all_engine_barrier()
        assert self.sems is not None
        popped = self.nc._tile_sem_poison_stack.pop()
        assert popped is self._sem_poison
        self.nc.clear_and_free_semaphores(list(self.sems.allocated().values()))
        self.nc.all_engine_barrier()

    tile_mod.TileContext._drain_and_barrier = _drain_and_barrier
    tile_mod.TileContext._drain_patched = True


_ENGINE_SEM_PREFIX = {
    "EngineType.PE": "PE_",
    "EngineType.DVE": "DVE_",
    "EngineType.Activation": "Activation_",
    "EngineType.Pool": "Pool_",
    "EngineType.SP": "SP_",
}


def _drop_self_waits(nc):
    """Remove waits on an instruction's OWN engine semaphore.  Such a wait
    references a tick from an earlier instruction on the same in-order engine
    (waiting on your own future would deadlock), so it is always satisfied by
    program order; the actual SRAM write completes while the engine is busy
    (only the ack return is pipelined), so same-engine RAW through memory is
    ordered too.  Dropping them keeps most instructions at one wait, which
    avoids SEQ-blocking wsplit NoOps."""
    n_drop = 0
    for fn in nc.m.functions:
        for bb in fn.blocks:
            for inst in bb.instructions:
                si = inst.sync_info
                if si is None or not si.on_wait:
                    continue
                pref = _ENGINE_SEM_PREFIX.get(str(inst.engine))
                if pref is None:
                    continue
                kept = [
                    w for w in si.on_wait
                    if not (w.ant_name or "").startswith(pref)
                ]
                if len(kept) != len(si.on_wait):
                    n_drop += len(si.on_wait) - len(kept)
                    si.on_wait = kept
    return n_drop


def _split_multiwaits(nc, mybir):
    """Walrus rejects instructions with more than one semaphore wait.  Hoist
    extra waits onto same-engine NoOps inserted just before the offender —
    semantically identical (the engine blocks on each wait in order)."""
    n_split = 0
    for fn in nc.m.functions:
        for bb in fn.blocks:
            insts = list(bb.instructions)
            out = []
            for inst in insts:
                si = inst.sync_info
                waits = list(si.on_wait) if si is not None and si.on_wait else []
                if len(waits) > 1:
                    for i, w in enumerate(waits[:-1]):
                        nop = mybir.InstNoOp(
                            name=f"{inst.name}-wsplit{i}",
                            engine=inst.engine,
                            sync_info=mybir.SyncInfo(on_wait=[w], on_update=[]),
                        )
                        out.append(nop)
                        n_split += 1
                    si.on_wait = [waits[-1]]
                out.append(inst)
            if len(out) != len(insts):
                bb.instructions = out
    return n_split


def _build_program(
    n_steps=L, k_chains=K, bl=BL, lag=D, q_p=Q_P, q_r=Q_R, q_s=Q_S,
    chunk_steps=CHUNK_STEPS, n_out_dma=N_OUT_DMA, split_waits=True,
):
    import concourse.bass as bass
    import concourse.mybir as mybir
    from concourse import tile
    from contextlib import ExitStack

    _patch_drain(tile)

    f32 = mybir.dt.float32
    bf16 = mybir.dt.bfloat16
    dt_p = bf16 if q_p else f32
    dt_r = bf16 if q_r else f32
    dt_s = bf16 if q_s else f32
    nc = bass.Bass()

    f = C * bl
    n_slots = n_steps * k_chains

    # pbuf carries a 128-col identity prefix (P-inject lhsT) so it rides the
    # first chunk DMA instead of needing its own serialized HWDGE slot.
    pbuf_d = nc.dram_tensor(
        "pbuf", [128, 128 + n_slots * f], dt_p, kind="ExternalInput"
    )
    wm_d = nc.dram_tensor("wm", [4, H], dt_s, kind="ExternalInput")
    wn_d = nc.dram_tensor("wn", [128, C * R], dt_r, kind="ExternalInput")
    aout_d = nc.dram_tensor("aout", [4, n_slots * bl], dt_s, kind="ExternalOutput")

    mult = mybir.AluOpType.mult
    add = mybir.AluOpType.add
    tanh_fn = mybir.ActivationFunctionType.Tanh

    with ExitStack() as ctx:
        tc = ctx.enter_context(tile.TileContext(nc))
        cpool = ctx.enter_context(tc.tile_pool(name="const", bufs=1))
        thpool = ctx.enter_context(tc.tile_pool(name="th", bufs=2))
        ph_pools = [
            ctx.enter_context(tc.tile_pool(name=f"ph{k}", bufs=1, space="PSUM"))
            for k in range(k_chains)
        ]
        pu_pools = [
            ctx.enter_context(tc.tile_pool(name=f"pu{k}", bufs=1, space="PSUM"))
            for k in range(k_chains)
        ]

        pbuf_s = cpool.tile([128, 128 + n_slots * f], dt_p, tag="pbuf")
        wm_s = cpool.tile([4, H], dt_s, tag="wm")
        wn_s = cpool.tile([128, C * R], dt_r, tag="wn")
        id_s = pbuf_s[:, 0:128]
        # state + history: slot (t+1)*K + k = A after step t of chain k;
        # slots 0..K-1 = initial zeros.
        z_s = cpool.tile([4, (n_slots + k_chains) * bl], dt_s, tag="z")

        # DMA issue order is start-latency-critical: each issue occupies its
        # queue's SEQ ~650ns.  Weights go on the ACT queue, pbuf chunks +
        # wn on the SP queue, so the first P-inject/expansion unblock ~2us
        # earlier than a single serialized queue.  The first chunks are
        # small so compute starts early.
        nc.scalar.dma_start(out=wm_s[:, :], in_=wm_d[:, :])
        plan, pos_steps = [], 0
        for w_ in (1, 1, 2, 4):
            if pos_steps + w_ <= n_steps:
                plan.append(w_)
                pos_steps += w_
        while pos_steps < n_steps:
            w_ = min(chunk_steps, n_steps - pos_steps)
            plan.append(w_)
            pos_steps += w_
        pos = 0
        for i, w_ in enumerate(plan):
            start_col = 0 if i == 0 else 128 + pos * k_chains * f
            pos += w_
            end_col = 128 + pos * k_chains * f
            nc.sync.dma_start(
                out=pbuf_s[:, start_col:end_col], in_=pbuf_d[:, start_col:end_col]
            )
            if i == 0:
                nc.sync.dma_start(out=wn_s[:, :], in_=wn_d[:, :])

        nc.vector.memset(z_s[:, 0 : k_chains * bl], 0.0)

        def p_inject(k, t):
            ph = ph_pools[k].tile([128, f], f32, tag=f"ph{k}")
            s = t * k_chains + k
            nc.tensor.matmul(
                ph[:, :],
                lhsT=id_s[:, :],
                rhs=pbuf_s[:, 128 + s * f : 128 + (s + 1) * f],
                start=True, stop=False, skip_group_check=True,
            )
            return ph

        def expansion(k, t, ph):
            s = t * k_chains + k
            rhs = z_s[0:4, s * bl : (s + 1) * bl]
            for c in range(C):
                nc.tensor.matmul(
                    ph[:, c * bl : (c + 1) * bl],
                    lhsT=wm_s[0:4, c * 128 : (c + 1) * 128],
                    rhs=rhs,
                    start=False, stop=(c == C - 1), skip_group_check=True,
                )

        def tanh_op(k, t, ph):
            th = thpool.tile([128, f], dt_r, tag=f"th{k}")
            nc.scalar.activation(th[:, :], ph[:, :], tanh_fn)
            return th

        def reduction(k, t, th):
            pu = pu_pools[k].tile([4, bl], f32, tag=f"pu{k}")
            for c in range(C):
                nc.tensor.matmul(
                    pu[0:4, 0:bl],
                    lhsT=wn_s[:, c * R : (c + 1) * R],
                    rhs=th[:, c * bl : (c + 1) * bl],
                    start=(c == 0), stop=(c == C - 1), skip_group_check=True,
                )
            return pu

        def a_copy(k, t, pu):
            s = t * k_chains + k
            nc.vector.scalar_tensor_tensor(
                out=z_s[0:4, (s + k_chains) * bl : (s + k_chains + 1) * bl],
                in0=z_s[0:4, s * bl : (s + 1) * bl],
                scalar=1.0 - ALPHA,
                in1=pu[0:4, 0:bl],
                op0=mult, op1=add,
            )

        # chain pipeline state
        ph_live = [None] * k_chains
        th_live = [None] * k_chains
        pu_live = [None] * k_chains

        for k in range(k_chains):
            ph_live[k] = p_inject(k, 0)

        def front(s):
            t, k = divmod(s, k_chains)
            expansion(k, t, ph_live[k])
            th_live[k] = tanh_op(k, t, ph_live[k])

        def back(s):
            t, k = divmod(s, k_chains)
            pu_live[k] = reduction(k, t, th_live[k])
            if t + 1 < n_steps:
                ph_live[k] = p_inject(k, t + 1)
            a_copy(k, t, pu_live[k])

        # streamed output: after every OC completed back()s, DMA the finished
        # z history range out (waits only on the last a_copy in the range)
        oc = max(1, (n_slots + n_out_dma - 1) // n_out_dma)
        out_pos = 0

        def drain_out(done):
            nonlocal out_pos
            while done - out_pos >= oc or (done == n_slots and out_pos < n_slots):
                end = min(out_pos + oc, n_slots)
                nc.sync.dma_start(
                    out=aout_d[:, out_pos * bl : end * bl],
                    in_=z_s[0:4, (k_chains + out_pos) * bl : (k_chains + end) * bl],
                )
                out_pos = end

        for s in range(n_slots):
            front(s)
            if s >= lag:
                back(s - lag)
                drain_out(s - lag + 1)
        for s in range(n_slots - lag, n_slots):
            back(s)
        drain_out(n_slots)

    _drop_self_waits(nc)
    if split_waits:
        _split_multiwaits(nc, mybir)
    return nc


def _get_program(t_steps=None, **kw):
    key = tuple(sorted(kw.items()))
    if key not in _CACHE:
        _CACHE[key] = _build_program(**kw)
    return _CACHE[key]


def _host_prep(x, m, n, I_mat, q_p=Q_P, q_r=Q_R, q_s=Q_S):
    """Per-core input maps + host-computed C EMA.

    Global chain id G = core*K + k maps to (seg, grp) = (G // N_G, G % N_G).
    pbuf layout per core: col ((t*K + k)*F + c*BL + b) on partition p holds
    P_{g-1}[grp*BL + b, c*128 + p] where g = seg*SEG - W + t (g<=0 -> 0).
    """
    x = np.asarray(x, np.float32)
    m = np.asarray(m, np.float32)
    n = np.asarray(n, np.float32)
    I_mat = np.asarray(I_mat, np.float32)

    wm = np.ascontiguousarray((ALPHA * SCALE * m).T)            # [4, H] f32
    wn = np.ascontiguousarray(
        n.reshape(C, 128, R).transpose(1, 0, 2).reshape(128, C * R)
    )
    idm = np.eye(128, dtype=np.float32)
    dt_p = ml_dtypes.bfloat16 if q_p else np.float32
    dt_r = ml_dtypes.bfloat16 if q_r else np.float32
    dt_s = ml_dtypes.bfloat16 if q_s else np.float32
    wn = wn.astype(dt_r)
    idm = idm.astype(dt_p)
    wm = wm.astype(dt_s)

    # Q[g] = P_{g-1}: P_g = 0.9 P_{g-1} + a * x_g @ I.T ; Q[0] = 0
    xI = ALPHA * np.matmul(x, I_mat.T)                          # [B, T, H]
    Q = np.zeros((B, T, H), np.float32)
    if T > 1:
        Q[:, 1] = xI[:, 0]
        for t in range(2, T):
            Q[:, t] = (1.0 - ALPHA) * Q[:, t - 1] + xI[:, t - 1]

    # C EMA on host (f64 accumulate, cast back)
    Ce = np.zeros((B, T, IN), np.float64)
    Ce[:, 0] = x[:, 0, :]
    for t in range(1, T):
        Ce[:, t] = (1.0 - ALPHA) * Ce[:, t - 1] + x[:, t, :]
    Ce = Ce.astype(np.float32)

    in_maps = []
    for core in range(N_CORES):
        chains = np.empty((128, L, K, C, BL), np.float32)
        for k in range(K):
            g = core * K + k
            seg, grp = divmod(g, N_G)
            g0 = SEG_START[seg] - SEG_WARM[seg]
            blk = Q[grp * BL : (grp + 1) * BL, g0 : g0 + L]
            # [BL, L, H] -> [128, L, C, BL]
            chains[:, :, k] = blk.reshape(BL, L, C, 128).transpose(3, 1, 2, 0)
        pbuf = np.ascontiguousarray(
            np.concatenate(
                [idm.astype(np.float32), chains.reshape(128, L * K * F)], axis=1
            ).astype(dt_p)
        )
        in_maps.append({"pbuf": pbuf, "wm": wm, "wn": wn})
    return in_maps, Ce


def _assemble(results, Ce):
    out = np.empty((B, T, R + IN), np.float32)
    for core in range(N_CORES):
        a = np.asarray(results[core]["aout"], np.float32)       # [4, L*K*BL]
        a = a.reshape(4, L, K, BL)
        for k in range(K):
            g = core * K + k
            seg, grp = divmod(g, N_G)
            w = SEG_WARM[seg]
            s0 = SEG_START[seg]
            keep = a[:, w:, k, :]                               # [4, L-w, BL]
            out[grp * BL : (grp + 1) * BL, s0 : s0 + L - w, 0:R] = (
                (ALPHA * SCALE) * keep.transpose(2, 1, 0)
            )
    out[:, :, R:] = ALPHA * Ce
    return out


def kernel(x, m, n, I_mat, _trace=False, **_build_kw):
    global LAST_RESULTS
    from concourse.bass_utils import run_bass_kernel_spmd

    nc = _get_program(**_build_kw)
    in_maps, Ce = _host_prep(
        x, m, n, I_mat,
        q_p=_build_kw.get("q_p", Q_P), q_r=_build_kw.get("q_r", Q_R),
        q_s=_build_kw.get("q_s", Q_S),
    )
    res = run_bass_kernel_spmd(
        nc, in_maps, core_ids=list(range(N_CORES)), trace=_trace
    )
    LAST_RESULTS = res
    return _assemble(res.results, Ce)


# revision 34
# speedup vs baseline: 1.0001x; 1.0001x over previous
"""Trainium2 Bass kernel for the MixtureLowRankRNN problem (v2).

Math background
---------------
Reference recurrence (per batch row, h0 = 0):
    h_t   = (1-a) h_{t-1} + a*scale*(m @ n.T tanh(h_{t-1})) + a*(x_t @ I.T)
    out_t = pinv([m, I]) @ h_t
Every update stays in colspan([m, I]), so h_t = m@(a*scale*A_t) + I@(a*C_t)
exactly, with
    A_t = (1-a) A_{t-1} + n.T tanh(h_{t-1})      (device: nonlinear scan)
    C_t = (1-a) C_{t-1} + x_t                    (host: x-only EMA)
and out_t = [a*scale*A_t ; a*C_t].  The scan uses the host-precomputed drive
    P_t = (1-a) P_{t-1} + a*(x_t @ I.T),  h_t = (a*scale*m)^T-exp(A_{t-1}) + P_{t-1}.

Sharding: the dynamics CONTRACT (~0.947/step: tanh is mostly in its linear
region, Jacobian ~ 0.9 I), so time segments warm-started W steps early from
A=0 converge to the true trajectory (numpy-verified; the full bf16 pipeline
at N_T=16, W=64 measures 7.4e-3 global rel err vs the 2e-2 gate).  N_T time
segments x N_G batch groups = 8 cores x K interleaved chains/core; segment 0
needs no warmup (its history IS exact), so it outputs all L steps while
segments 1.. output L-W: L = (T + (N_T-1) W) / N_T = 124 device steps.

Per-chain step (full unroll; global slot s = t*K + k on each core):
  PE : P-inject matmul (identity bf16 @ pbuf slot) -> opens ph(k) bank,
       sets has_written; 8 expansion mms (wm bf16 x A bf16) accumulate on top.
  ACT: tanh ph(k) -> th (bf16)
  PE : 8 reduction mms (wn bf16 x th bf16) -> pu(k) bank
  DVE: a_copy STT  z[(t+1)K+k] = 0.9*z[tK+k] + pu(k)   (state + history)
The reduction/a-copy/next P-inject of a chain are emitted D = K//2 slots
after its expansion (software pipelining), so the in-order engines never
block across chains; the steady cycle sits at the per-chain latency floor
(~1377ns: mm ack 173x2 + ACT 292+185 + DVE 142+125 + sem hops).  PSUM:
K x (ph + pu) = 2K banks (= 8 at K=4; one accumulation group per bank since
start=True clears a whole bank's has_written bits).  The identity rides as a
128-col prefix of pbuf so it needs no separate serialized DMA; pbuf preloads
in small-then-large chunks overlapped with compute, and the A history is
DMA'd out in chunks as it becomes final.
"""

import numpy as np
import ml_dtypes

H = 1024
R = 4
IN = 16
ALPHA = 0.1
SCALE = 500.0 / 1024.0
B = 32
T = 1024
N_CORES = 8
C = H // 128

# ---- configuration ---------------------------------------------------------
BL = 16                   # batch rows per chain
N_G = B // BL             # batch groups (2)
N_T = 16                  # time segments
K = (N_T * N_G) // N_CORES  # chains per core (4)
W = 64                    # warmup steps per segment (segment 0 needs none)
# Uneven segments: segment 0 outputs all L steps (its warmup is exact anyway);
# segments 1.. warm up W steps and output L-W.  L + (N_T-1)(L-W) = T.
L = (T + (N_T - 1) * W) // N_T   # device steps per chain (124)
assert L + (N_T - 1) * (L - W) == T
# per-segment (global output start, warmup steps)
SEG_START = [0] + [L + (j - 1) * (L - W) for j in range(1, N_T)]
SEG_WARM = [0] + [W] * (N_T - 1)
F = C * BL                # ph free width (128)
D = max(K // 2, 0)        # software-pipeline lag (slots)
Q_P = True                # pbuf (P drive) in bf16
Q_R = True                # reduction path (wn, th) in bf16
Q_S = True                # state path (z, wm, aout) in bf16
CHUNK_STEPS = 12          # pbuf DMA chunk granularity (steps)
N_OUT_DMA = 16            # streamed output DMA chunks

_CACHE = {}
LAST_RESULTS = None


def _patch_drain(tile_mod):
    """This walrus build allows only ONE semaphore wait per instruction, but
    TileContext's tail drain waits on every active proc.  Split it: one
    single-wait NOP per proc, then a waitless drain."""
    if getattr(tile_mod.TileContext, "_drain_patched", False):
        return
    from concourse.vector_clock import ScopedClock, VectorClock

    def _drain_and_barrier(self, tick_clock, wait_clock):
        gc = tick_clock.global_clock
        n = len(gc)
        for proc in range(n):
            t = gc[proc]
            if t > 0:
                nop_inst = self.nc.sync.nop()
                vec = [0] * n
                vec[proc] = t
                wait_clock.add_sem_waits(
                    nop_inst.ins, ScopedClock({None: VectorClock(vec)})
                )
        self.nc.sync.drain()
        self.nc.all_engine_barrier()
        assert self.sems is not None
        popped = self.nc._tile_sem_poison_stack.pop()
        assert popped is self._sem_poison
        self.nc.clear_and_free_semaphores(list(self.sems.allocated().values()))
        self.nc.all_engine_barrier()

    tile_mod.TileContext._drain_and_barrier = _drain_and_barrier
    tile_mod.TileContext._drain_patched = True


_ENGINE_SEM_PREFIX = {
    "EngineType.PE": "PE_",
    "EngineType.DVE": "DVE_",
    "EngineType.Activation": "Activation_",
    "EngineType.Pool": "Pool_",
    "EngineType.SP": "SP_",
}


def _drop_self_waits(nc):
    """Remove waits on an instruction's OWN engine semaphore.  Such a wait
    references a tick from an earlier instruction on the same in-order engine
    (waiting on your own future would deadlock), so it is always satisfied by
    program order; the actual SRAM write completes while the engine is busy
    (only the ack return is pipelined), so same-engine RAW through memory is
    ordered too.  Dropping them keeps most instructions at one wait, which
    avoids SEQ-blocking wsplit NoOps."""
    n_drop = 0
    for fn in nc.m.functions:
        for bb in fn.blocks:
            for inst in bb.instructions:
                si = inst.sync_info
                if si is None or not si.on_wait:
                    continue
                pref = _ENGINE_SEM_PREFIX.get(str(inst.engine))
                if pref is None:
                    continue
                kept = [
                    w for w in si.on_wait
                    if not (w.ant_name or "").startswith(pref)
                ]
                if len(kept) != len(si.on_wait):
                    n_drop += len(si.on_wait) - len(kept)
                    si.on_wait = kept
    return n_drop


def _split_multiwaits(nc, mybir):
    """Walrus rejects instructions with more than one semaphore wait.  Hoist
    extra waits onto same-engine NoOps inserted just before the offender —
    semantically identical (the engine blocks on each wait in order)."""
    n_split = 0
    for fn in nc.m.functions:
        for bb in fn.blocks:
            insts = list(bb.instructions)
            out = []
            for inst in insts:
                si = inst.sync_info
                waits = list(si.on_wait) if si is not None and si.on_wait else []
                if len(waits) > 1:
                    for i, w in enumerate(waits[:-1]):
                        nop = mybir.InstNoOp(
                            name=f"{inst.name}-wsplit{i}",
                            engine=inst.engine,
                            sync_info=mybir.SyncInfo(on_wait=[w], on_update=[]),
                        )
                        out.append(nop)
                        n_split += 1
                    si.on_wait = [waits[-1]]
                out.append(inst)
            if len(out) != len(insts):
                bb.instructions = out
    return n_split


def _build_program(
    n_steps=L, k_chains=K, bl=BL, lag=D, q_p=Q_P, q_r=Q_R, q_s=Q_S,
    chunk_steps=CHUNK_STEPS, n_out_dma=N_OUT_DMA, split_waits=True,
):
    import concourse.bass as bass
    import concourse.mybir as mybir
    from concourse import tile
    from contextlib import ExitStack

    _patch_drain(tile)

    f32 = mybir.dt.float32
    bf16 = mybir.dt.bfloat16
    dt_p = bf16 if q_p else f32
    dt_r = bf16 if q_r else f32
    dt_s = bf16 if q_s else f32
    nc = bass.Bass()

    f = C * bl
    n_slots = n_steps * k_chains

    # pbuf carries a 128-col identity prefix (P-inject lhsT) so it rides the
    # first chunk DMA instead of needing its own serialized HWDGE slot.
    pbuf_d = nc.dram_tensor(
        "pbuf", [128, 128 + n_slots * f], dt_p, kind="ExternalInput"
    )
    wm_d = nc.dram_tensor("wm", [4, H], dt_s, kind="ExternalInput")
    wn_d = nc.dram_tensor("wn", [128, C * R], dt_r, kind="ExternalInput")
    aout_d = nc.dram_tensor("aout", [4, n_slots * bl], dt_s, kind="ExternalOutput")

    mult = mybir.AluOpType.mult
    add = mybir.AluOpType.add
    tanh_fn = mybir.ActivationFunctionType.Tanh

    with ExitStack() as ctx:
        tc = ctx.enter_context(tile.TileContext(nc))
        cpool = ctx.enter_context(tc.tile_pool(name="const", bufs=1))
        thpool = ctx.enter_context(tc.tile_pool(name="th", bufs=2))
        ph_pools = [
            ctx.enter_context(tc.tile_pool(name=f"ph{k}", bufs=1, space="PSUM"))
            for k in range(k_chains)
        ]
        pu_pools = [
            ctx.enter_context(tc.tile_pool(name=f"pu{k}", bufs=1, space="PSUM"))
            for k in range(k_chains)
        ]

        pbuf_s = cpool.tile([128, 128 + n_slots * f], dt_p, tag="pbuf")
        wm_s = cpool.tile([4, H], dt_s, tag="wm")
        wn_s = cpool.tile([128, C * R], dt_r, tag="wn")
        id_s = pbuf_s[:, 0:128]
        # state + history: slot (t+1)*K + k = A after step t of chain k;
        # slots 0..K-1 = initial zeros.
        z_s = cpool.tile([4, (n_slots + k_chains) * bl], dt_s, tag="z")

        # DMA issue order is start-latency-critical: each issue occupies its
        # queue's SEQ ~650ns.  Weights go on the ACT queue, pbuf chunks +
        # wn on the SP queue, so the first P-inject/expansion unblock ~2us
        # earlier than a single serialized queue.  The first chunks are
        # small so compute starts early.
        nc.scalar.dma_start(out=wm_s[:, :], in_=wm_d[:, :])
        plan, pos_steps = [], 0
        for w_ in (1, 1, 2, 4):
            if pos_steps + w_ <= n_steps:
                plan.append(w_)
                pos_steps += w_
        while pos_steps < n_steps:
            w_ = min(chunk_steps, n_steps - pos_steps)
            plan.append(w_)
            pos_steps += w_
        pos = 0
        for i, w_ in enumerate(plan):
            start_col = 0 if i == 0 else 128 + pos * k_chains * f
            pos += w_
            end_col = 128 + pos * k_chains * f
            nc.sync.dma_start(
                out=pbuf_s[:, start_col:end_col], in_=pbuf_d[:, start_col:end_col]
            )
            if i == 0:
                nc.sync.dma_start(out=wn_s[:, :], in_=wn_d[:, :])

        nc.vector.memset(z_s[:, 0 : k_chains * bl], 0.0)

        def p_inject(k, t):
            ph = ph_pools[k].tile([128, f], f32, tag=f"ph{k}")
            s = t * k_chains + k
            nc.tensor.matmul(
                ph[:, :],
                lhsT=id_s[:, :],
                rhs=pbuf_s[:, 128 + s * f : 128 + (s + 1) * f],
                start=True, stop=False, skip_group_check=True,
            )
            return ph

        def expansion(k, t, ph):
            s = t * k_chains + k
            rhs = z_s[0:4, s * bl : (s + 1) * bl]
            for c in range(C):
                nc.tensor.matmul(
                    ph[:, c * bl : (c + 1) * bl],
                    lhsT=wm_s[0:4, c * 128 : (c + 1) * 128],
                    rhs=rhs,
                    start=False, stop=(c == C - 1), skip_group_check=True,
                )

        def tanh_op(k, t, ph):
            th = thpool.tile([128, f], dt_r, tag=f"th{k}")
            nc.scalar.activation(th[:, :], ph[:, :], tanh_fn)
            return th

        def reduction(k, t, th):
            pu = pu_pools[k].tile([4, bl], f32, tag=f"pu{k}")
            for c in range(C):
                nc.tensor.matmul(
                    pu[0:4, 0:bl],
                    lhsT=wn_s[:, c * R : (c + 1) * R],
                    rhs=th[:, c * bl : (c + 1) * bl],
                    start=(c == 0), stop=(c == C - 1), skip_group_check=True,
                )
            return pu

        def a_copy(k, t, pu):
            s = t * k_chains + k
            nc.vector.scalar_tensor_tensor(
                out=z_s[0:4, (s + k_chains) * bl : (s + k_chains + 1) * bl],
                in0=z_s[0:4, s * bl : (s + 1) * bl],
                scalar=1.0 - ALPHA,
                in1=pu[0:4, 0:bl],
                op0=mult, op1=add,
            )

        # chain pipeline state
        ph_live = [None] * k_chains
        th_live = [None] * k_chains
        pu_live = [None] * k_chains

        for k in range(k_chains):
            ph_live[k] = p_inject(k, 0)

        def front(s):
            t, k = divmod(s, k_chains)
            expansion(k, t, ph_live[k])
            th_live[k] = tanh_op(k, t, ph_live[k])

        def back(s):
            t, k = divmod(s, k_chains)
            pu_live[k] = reduction(k, t, th_live[k])
            if t + 1 < n_steps:
                ph_live[k] = p_inject(k, t + 1)
            a_copy(k, t, pu_live[k])

        # streamed output: after every OC completed back()s, DMA the finished
        # z history range out (waits only on the last a_copy in the range)
        oc = max(1, (n_slots + n_out_dma - 1) // n_out_dma)
        out_pos = 0

        def drain_out(done):
            nonlocal out_pos
            while done - out_pos >= oc or (done == n_slots and out_pos < n_slots):
                end = min(out_pos + oc, n_slots)
                nc.sync.dma_start(
                    out=aout_d[:, out_pos * bl : end * bl],
                    in_=z_s[0:4, (k_chains + out_pos) * bl : (k_chains + end) * bl],
                )
                out_pos = end

        for s in range(n_slots):
            front(s)
            if s >= lag:
                back(s - lag)
                drain_out(s - lag + 1)
        for s in range(n_slots - lag, n_slots):
            back(s)
        drain_out(n_slots)

    _drop_self_waits(nc)
    if split_waits:
        _split_multiwaits(nc, mybir)
    return nc


def _get_program(t_steps=None, **kw):
    key = tuple(sorted(kw.items()))
    if key not in _CACHE:
        _CACHE[key] = _build_program(**kw)
    return _CACHE[key]


def _host_prep(x, m, n, I_mat, q_p=Q_P, q_r=Q_R, q_s=Q_S):
    """Per-core input maps + host-computed C EMA.

    Global chain id G = core*K + k maps to (seg, grp) = (G // N_G, G % N_G).
    pbuf layout per core: col ((t*K + k)*F + c*BL + b) on partition p holds
    P_{g-1}[grp*BL + b, c*128 + p] where g = seg*SEG - W + t (g<=0 -> 0).
    """
    x = np.asarray(x, np.float32)
    m = np.asarray(m, np.float32)
    n = np.asarray(n, np.float32)
    I_mat = np.asarray(I_mat, np.float32)

    wm = np.ascontiguousarray((ALPHA * SCALE * m).T)            # [4, H] f32
    wn = np.ascontiguousarray(
        n.reshape(C, 128, R).transpose(1, 0, 2).reshape(128, C * R)
    )
    idm = np.eye(128, dtype=np.float32)
    dt_p = ml_dtypes.bfloat16 if q_p else np.float32
    dt_r = ml_dtypes.bfloat16 if q_r else np.float32
    dt_s = ml_dtypes.bfloat16 if q_s else np.float32
    wn = wn.astype(dt_r)
    idm = idm.astype(dt_p)
    wm = wm.astype(dt_s)

    # Q[g] = P_{g-1}: P_g = 0.9 P_{g-1} + a * x_g @ I.T ; Q[0] = 0
    xI = ALPHA * np.matmul(x, I_mat.T)                          # [B, T, H]
    Q = np.zeros((B, T, H), np.float32)
    if T > 1:
        Q[:, 1] = xI[:, 0]
        for t in range(2, T):
            Q[:, t] = (1.0 - ALPHA) * Q[:, t - 1] + xI[:, t - 1]

    # C EMA on host (f64 accumulate, cast back)
    Ce = np.zeros((B, T, IN), np.float64)
    Ce[:, 0] = x[:, 0, :]
    for t in range(1, T):
        Ce[:, t] = (1.0 - ALPHA) * Ce[:, t - 1] + x[:, t, :]
    Ce = Ce.astype(np.float32)

    in_maps = []
    for core in range(N_CORES):
        chains = np.empty((128, L, K, C, BL), np.float32)
        for k in range(K):
            g = core * K + k
            seg, grp = divmod(g, N_G)
            g0 = SEG_START[seg] - SEG_WARM[seg]
            blk = Q[grp * BL : (grp + 1) * BL, g0 : g0 + L]
            # [BL, L, H] -> [128, L, C, BL]
            chains[:, :, k] = blk.reshape(BL, L, C, 128).transpose(3, 1, 2, 0)
        pbuf = np.ascontiguousarray(
            np.concatenate(
                [idm.astype(np.float32), chains.reshape(128, L * K * F)], axis=1
            ).astype(dt_p)
        )
        in_maps.append({"pbuf": pbuf, "wm": wm, "wn": wn})
    return in_maps, Ce


def _assemble(results, Ce):
    out = np.empty((B, T, R + IN), np.float32)
    for core in range(N_CORES):
        a = np.asarray(results[core]["aout"], np.float32)       # [4, L*K*BL]
        a = a.reshape(4, L, K, BL)
        for k in range(K):
            g = core * K + k
            seg, grp = divmod(g, N_G)
            w = SEG_WARM[seg]
            s0 = SEG_START[seg]
            keep = a[:, w:, k, :]                               # [4, L-w, BL]
            out[grp * BL : (grp + 1) * BL, s0 : s0 + L - w, 0:R] = (
                (ALPHA * SCALE) * keep.transpose(2, 1, 0)
            )
    out[:, :, R:] = ALPHA * Ce
    return out


def kernel(x, m, n, I_mat, _trace=False, **_build_kw):
    global LAST_RESULTS
    from concourse.bass_utils import run_bass_kernel_spmd

    nc = _get_program(**_build_kw)
    in_maps, Ce = _host_prep(
        x, m, n, I_mat,
        q_p=_build_kw.get("q_p", Q_P), q_r=_build_kw.get("q_r", Q_R),
        q_s=_build_kw.get("q_s", Q_S),
    )
    res = run_bass_kernel_spmd(
        nc, in_maps, core_ids=list(range(N_CORES)), trace=_trace
    )
    LAST_RESULTS = res
    return _assemble(res.results, Ce)


# revision 38
# speedup vs baseline: 1.0004x; 1.0004x over previous
"""Trainium2 Bass kernel for the MixtureLowRankRNN problem (v2).

Math background
---------------
Reference recurrence (per batch row, h0 = 0):
    h_t   = (1-a) h_{t-1} + a*scale*(m @ n.T tanh(h_{t-1})) + a*(x_t @ I.T)
    out_t = pinv([m, I]) @ h_t
Every update stays in colspan([m, I]), so h_t = m@(a*scale*A_t) + I@(a*C_t)
exactly, with
    A_t = (1-a) A_{t-1} + n.T tanh(h_{t-1})      (device: nonlinear scan)
    C_t = (1-a) C_{t-1} + x_t                    (host: x-only EMA)
and out_t = [a*scale*A_t ; a*C_t].  The scan uses the host-precomputed drive
    P_t = (1-a) P_{t-1} + a*(x_t @ I.T),  h_t = (a*scale*m)^T-exp(A_{t-1}) + P_{t-1}.

Sharding: the dynamics CONTRACT (~0.947/step: tanh is mostly in its linear
region, Jacobian ~ 0.9 I), so time segments warm-started W steps early from
A=0 converge to the true trajectory (numpy-verified; the full bf16 pipeline
at N_T=16, W=64 measures 7.4e-3 global rel err vs the 2e-2 gate).  N_T time
segments x N_G batch groups = 8 cores x K interleaved chains/core; segment 0
needs no warmup (its history IS exact), so it outputs all L steps while
segments 1.. output L-W: L = (T + (N_T-1) W) / N_T = 124 device steps.

Per-chain step (full unroll; global slot s = t*K + k on each core):
  PE : P-inject matmul (identity bf16 @ pbuf slot) -> opens ph(k) bank,
       sets has_written; 8 expansion mms (wm bf16 x A bf16) accumulate on top.
  ACT: tanh ph(k) -> th (bf16)
  PE : 8 reduction mms (wn bf16 x th bf16) -> pu(k) bank
  DVE: a_copy STT  z[(t+1)K+k] = 0.9*z[tK+k] + pu(k)   (state + history)
The reduction/a-copy/next P-inject of a chain are emitted D = K//2 slots
after its expansion (software pipelining), so the in-order engines never
block across chains; the steady cycle sits at the per-chain latency floor
(~1377ns: mm ack 173x2 + ACT 292+185 + DVE 142+125 + sem hops).  PSUM:
K x (ph + pu) = 2K banks (= 8 at K=4; one accumulation group per bank since
start=True clears a whole bank's has_written bits).  The identity rides as a
128-col prefix of pbuf so it needs no separate serialized DMA; pbuf preloads
in small-then-large chunks overlapped with compute, and the A history is
DMA'd out in chunks as it becomes final.
"""

import numpy as np
import ml_dtypes

H = 1024
R = 4
IN = 16
ALPHA = 0.1
SCALE = 500.0 / 1024.0
B = 32
T = 1024
N_CORES = 8
C = H // 128

# ---- configuration ---------------------------------------------------------
BL = 16                   # batch rows per chain
N_G = B // BL             # batch groups (2)
N_T = 16                  # time segments
K = (N_T * N_G) // N_CORES  # chains per core (4)
W = 64                    # warmup steps per segment (segment 0 needs none)
# Uneven segments: segment 0 outputs all L steps (its warmup is exact anyway);
# segments 1.. warm up W steps and output L-W.  L + (N_T-1)(L-W) = T.
L = (T + (N_T - 1) * W) // N_T   # device steps per chain (124)
assert L + (N_T - 1) * (L - W) == T
# per-segment (global output start, warmup steps)
SEG_START = [0] + [L + (j - 1) * (L - W) for j in range(1, N_T)]
SEG_WARM = [0] + [W] * (N_T - 1)
F = C * BL                # ph free width (128)
D = max(K // 2, 0)        # software-pipeline lag (slots)
Q_P = True                # pbuf (P drive) in bf16
Q_R = True                # reduction path (wn, th) in bf16
Q_S = True                # state path (z, wm, aout) in bf16
CHUNK_STEPS = 12          # pbuf DMA chunk granularity (steps)
N_OUT_DMA = 16            # streamed output DMA chunks

_CACHE = {}
LAST_RESULTS = None


def _patch_drain(tile_mod):
    """This walrus build allows only ONE semaphore wait per instruction, but
    TileContext's tail drain waits on every active proc.  Split it: one
    single-wait NOP per proc, then a waitless drain."""
    if getattr(tile_mod.TileContext, "_drain_patched", False):
        return
    from concourse.vector_clock import ScopedClock, VectorClock

    def _drain_and_barrier(self, tick_clock, wait_clock):
        gc = tick_clock.global_clock
        n = len(gc)
        for proc in range(n):
            t = gc[proc]
            if t > 0:
                nop_inst = self.nc.sync.nop()
                vec = [0] * n
                vec[proc] = t
                wait_clock.add_sem_waits(
                    nop_inst.ins, ScopedClock({None: VectorClock(vec)})
                )
        self.nc.sync.drain()
        self.nc.all_engine_barrier()
        assert self.sems is not None
        popped = self.nc._tile_sem_poison_stack.pop()
        assert popped is self._sem_poison
        self.nc.clear_and_free_semaphores(list(self.sems.allocated().values()))
        self.nc.all_engine_barrier()

    tile_mod.TileContext._drain_and_barrier = _drain_and_barrier
    tile_mod.TileContext._drain_patched = True


_ENGINE_SEM_PREFIX = {
    "EngineType.PE": "PE_",
    "EngineType.DVE": "DVE_",
    "EngineType.Activation": "Activation_",
    "EngineType.Pool": "Pool_",
    "EngineType.SP": "SP_",
}


def _drop_self_waits(nc):
    """Remove waits on an instruction's OWN engine semaphore.  Such a wait
    references a tick from an earlier instruction on the same in-order engine
    (waiting on your own future would deadlock), so it is always satisfied by
    program order; the actual SRAM write completes while the engine is busy
    (only the ack return is pipelined), so same-engine RAW through memory is
    ordered too.  Dropping them keeps most instructions at one wait, which
    avoids SEQ-blocking wsplit NoOps."""
    n_drop = 0
    for fn in nc.m.functions:
        for bb in fn.blocks:
            for inst in bb.instructions:
                si = inst.sync_info
                if si is None or not si.on_wait:
                    continue
                pref = _ENGINE_SEM_PREFIX.get(str(inst.engine))
                if pref is None:
                    continue
                kept = [
                    w for w in si.on_wait
                    if not (w.ant_name or "").startswith(pref)
                ]
                if len(kept) != len(si.on_wait):
                    n_drop += len(si.on_wait) - len(kept)
                    si.on_wait = kept
    return n_drop


def _split_multiwaits(nc, mybir):
    """Walrus rejects instructions with more than one semaphore wait.  Hoist
    extra waits onto same-engine NoOps inserted just before the offender —
    semantically identical (the engine blocks on each wait in order)."""
    n_split = 0
    for fn in nc.m.functions:
        for bb in fn.blocks:
            insts = list(bb.instructions)
            out = []
            for inst in insts:
                si = inst.sync_info
                waits = list(si.on_wait) if si is not None and si.on_wait else []
                if len(waits) > 1:
                    for i, w in enumerate(waits[:-1]):
                        nop = mybir.InstNoOp(
                            name=f"{inst.name}-wsplit{i}",
                            engine=inst.engine,
                            sync_info=mybir.SyncInfo(on_wait=[w], on_update=[]),
                        )
                        out.append(nop)
                        n_split += 1
                    si.on_wait = [waits[-1]]
                out.append(inst)
            if len(out) != len(insts):
                bb.instructions = out
    return n_split


def _build_program(
    n_steps=L, k_chains=K, bl=BL, lag=D, q_p=Q_P, q_r=Q_R, q_s=Q_S,
    chunk_steps=CHUNK_STEPS, n_out_dma=N_OUT_DMA, split_waits=True,
):
    import concourse.bass as bass
    import concourse.mybir as mybir
    from concourse import tile
    from contextlib import ExitStack

    _patch_drain(tile)

    f32 = mybir.dt.float32
    bf16 = mybir.dt.bfloat16
    dt_p = bf16 if q_p else f32
    dt_r = bf16 if q_r else f32
    dt_s = bf16 if q_s else f32
    nc = bass.Bass()

    f = C * bl
    n_slots = n_steps * k_chains

    # pbuf carries a 128-col identity prefix (P-inject lhsT) so it rides the
    # first chunk DMA instead of needing its own serialized HWDGE slot.
    pbuf_d = nc.dram_tensor(
        "pbuf", [128, 128 + n_slots * f], dt_p, kind="ExternalInput"
    )
    wm_d = nc.dram_tensor("wm", [4, H], dt_s, kind="ExternalInput")
    wn_d = nc.dram_tensor("wn", [128, C * R], dt_r, kind="ExternalInput")
    aout_d = nc.dram_tensor("aout", [4, n_slots * bl], dt_s, kind="ExternalOutput")

    mult = mybir.AluOpType.mult
    add = mybir.AluOpType.add
    tanh_fn = mybir.ActivationFunctionType.Tanh

    with ExitStack() as ctx:
        tc = ctx.enter_context(tile.TileContext(nc))
        cpool = ctx.enter_context(tc.tile_pool(name="const", bufs=1))
        thpool = ctx.enter_context(tc.tile_pool(name="th", bufs=2))
        ph_pools = [
            ctx.enter_context(tc.tile_pool(name=f"ph{k}", bufs=1, space="PSUM"))
            for k in range(k_chains)
        ]
        pu_pools = [
            ctx.enter_context(tc.tile_pool(name=f"pu{k}", bufs=1, space="PSUM"))
            for k in range(k_chains)
        ]

        pbuf_s = cpool.tile([128, 128 + n_slots * f], dt_p, tag="pbuf")
        wm_s = cpool.tile([4, H], dt_s, tag="wm")
        wn_s = cpool.tile([128, C * R], dt_r, tag="wn")
        id_s = pbuf_s[:, 0:128]
        # state + history: slot (t+1)*K + k = A after step t of chain k;
        # slots 0..K-1 = initial zeros.
        z_s = cpool.tile([4, (n_slots + k_chains) * bl], dt_s, tag="z")

        # DMA issue order is start-latency-critical: each issue occupies its
        # queue's SEQ ~650ns.  Weights go on the ACT queue, pbuf chunks +
        # wn on the SP queue, so the first P-inject/expansion unblock ~2us
        # earlier than a single serialized queue.  The first chunks are
        # small so compute starts early.
        nc.scalar.dma_start(out=wm_s[:, :], in_=wm_d[:, :])
        plan, pos_steps = [], 0
        for w_ in (1, 1, 2, 4):
            if pos_steps + w_ <= n_steps:
                plan.append(w_)
                pos_steps += w_
        while pos_steps < n_steps:
            w_ = min(chunk_steps, n_steps - pos_steps)
            plan.append(w_)
            pos_steps += w_
        pos = 0
        for i, w_ in enumerate(plan):
            start_col = 0 if i == 0 else 128 + pos * k_chains * f
            pos += w_
            end_col = 128 + pos * k_chains * f
            nc.sync.dma_start(
                out=pbuf_s[:, start_col:end_col], in_=pbuf_d[:, start_col:end_col]
            )
            if i == 1:
                nc.sync.dma_start(out=wn_s[:, :], in_=wn_d[:, :])

        nc.vector.memset(z_s[:, 0 : k_chains * bl], 0.0)

        def p_inject(k, t):
            ph = ph_pools[k].tile([128, f], f32, tag=f"ph{k}")
            s = t * k_chains + k
            nc.tensor.matmul(
                ph[:, :],
                lhsT=id_s[:, :],
                rhs=pbuf_s[:, 128 + s * f : 128 + (s + 1) * f],
                start=True, stop=False, skip_group_check=True,
            )
            return ph

        def expansion(k, t, ph):
            s = t * k_chains + k
            rhs = z_s[0:4, s * bl : (s + 1) * bl]
            for c in range(C):
                nc.tensor.matmul(
                    ph[:, c * bl : (c + 1) * bl],
                    lhsT=wm_s[0:4, c * 128 : (c + 1) * 128],
                    rhs=rhs,
                    start=False, stop=(c == C - 1), skip_group_check=True,
                )

        def tanh_op(k, t, ph):
            th = thpool.tile([128, f], dt_r, tag=f"th{k}")
            nc.scalar.activation(th[:, :], ph[:, :], tanh_fn)
            return th

        def reduction(k, t, th):
            pu = pu_pools[k].tile([4, bl], f32, tag=f"pu{k}")
            for c in range(C):
                nc.tensor.matmul(
                    pu[0:4, 0:bl],
                    lhsT=wn_s[:, c * R : (c + 1) * R],
                    rhs=th[:, c * bl : (c + 1) * bl],
                    start=(c == 0), stop=(c == C - 1), skip_group_check=True,
                )
            return pu

        def a_copy(k, t, pu):
            s = t * k_chains + k
            nc.vector.scalar_tensor_tensor(
                out=z_s[0:4, (s + k_chains) * bl : (s + k_chains + 1) * bl],
                in0=z_s[0:4, s * bl : (s + 1) * bl],
                scalar=1.0 - ALPHA,
                in1=pu[0:4, 0:bl],
                op0=mult, op1=add,
            )

        # chain pipeline state
        ph_live = [None] * k_chains
        th_live = [None] * k_chains
        pu_live = [None] * k_chains

        for k in range(k_chains):
            ph_live[k] = p_inject(k, 0)

        def front(s):
            t, k = divmod(s, k_chains)
            expansion(k, t, ph_live[k])
            th_live[k] = tanh_op(k, t, ph_live[k])

        def back(s):
            t, k = divmod(s, k_chains)
            pu_live[k] = reduction(k, t, th_live[k])
            if t + 1 < n_steps:
                ph_live[k] = p_inject(k, t + 1)
            a_copy(k, t, pu_live[k])

        # streamed output: after every OC completed back()s, DMA the finished
        # z history range out (waits only on the last a_copy in the range)
        oc = max(1, (n_slots + n_out_dma - 1) // n_out_dma)
        out_pos = 0

        def drain_out(done):
            nonlocal out_pos
            while done - out_pos >= oc or (done == n_slots and out_pos < n_slots):
                end = min(out_pos + oc, n_slots)
                nc.sync.dma_start(
                    out=aout_d[:, out_pos * bl : end * bl],
                    in_=z_s[0:4, (k_chains + out_pos) * bl : (k_chains + end) * bl],
                )
                out_pos = end

        for s in range(n_slots):
            front(s)
            if s >= lag:
                back(s - lag)
                drain_out(s - lag + 1)
        for s in range(n_slots - lag, n_slots):
            back(s)
        drain_out(n_slots)

    _drop_self_waits(nc)
    if split_waits:
        _split_multiwaits(nc, mybir)
    return nc


def _get_program(t_steps=None, **kw):
    key = tuple(sorted(kw.items()))
    if key not in _CACHE:
        _CACHE[key] = _build_program(**kw)
    return _CACHE[key]


def _host_prep(x, m, n, I_mat, q_p=Q_P, q_r=Q_R, q_s=Q_S):
    """Per-core input maps + host-computed C EMA.

    Global chain id G = core*K + k maps to (seg, grp) = (G // N_G, G % N_G).
    pbuf layout per core: col ((t*K + k)*F + c*BL + b) on partition p holds
    P_{g-1}[grp*BL + b, c*128 + p] where g = seg*SEG - W + t (g<=0 -> 0).
    """
    x = np.asarray(x, np.float32)
    m = np.asarray(m, np.float32)
    n = np.asarray(n, np.float32)
    I_mat = np.asarray(I_mat, np.float32)

    wm = np.ascontiguousarray((ALPHA * SCALE * m).T)            # [4, H] f32
    wn = np.ascontiguousarray(
        n.reshape(C, 128, R).transpose(1, 0, 2).reshape(128, C * R)
    )
    idm = np.eye(128, dtype=np.float32)
    dt_p = ml_dtypes.bfloat16 if q_p else np.float32
    dt_r = ml_dtypes.bfloat16 if q_r else np.float32
    dt_s = ml_dtypes.bfloat16 if q_s else np.float32
    wn = wn.astype(dt_r)
    idm = idm.astype(dt_p)
    wm = wm.astype(dt_s)

    # Q[g] = P_{g-1}: P_g = 0.9 P_{g-1} + a * x_g @ I.T ; Q[0] = 0
    xI = ALPHA * np.matmul(x, I_mat.T)                          # [B, T, H]
    Q = np.zeros((B, T, H), np.float32)
    if T > 1:
        Q[:, 1] = xI[:, 0]
        for t in range(2, T):
            Q[:, t] = (1.0 - ALPHA) * Q[:, t - 1] + xI[:, t - 1]

    # C EMA on host (f64 accumulate, cast back)
    Ce = np.zeros((B, T, IN), np.float64)
    Ce[:, 0] = x[:, 0, :]
    for t in range(1, T):
        Ce[:, t] = (1.0 - ALPHA) * Ce[:, t - 1] + x[:, t, :]
    Ce = Ce.astype(np.float32)

    in_maps = []
    for core in range(N_CORES):
        chains = np.empty((128, L, K, C, BL), np.float32)
        for k in range(K):
            g = core * K + k
            seg, grp = divmod(g, N_G)
            g0 = SEG_START[seg] - SEG_WARM[seg]
            blk = Q[grp * BL : (grp + 1) * BL, g0 : g0 + L]
            # [BL, L, H] -> [128, L, C, BL]
            chains[:, :, k] = blk.reshape(BL, L, C, 128).transpose(3, 1, 2, 0)
        pbuf = np.ascontiguousarray(
            np.concatenate(
                [idm.astype(np.float32), chains.reshape(128, L * K * F)], axis=1
            ).astype(dt_p)
        )
        in_maps.append({"pbuf": pbuf, "wm": wm, "wn": wn})
    return in_maps, Ce


def _assemble(results, Ce):
    out = np.empty((B, T, R + IN), np.float32)
    for core in range(N_CORES):
        a = np.asarray(results[core]["aout"], np.float32)       # [4, L*K*BL]
        a = a.reshape(4, L, K, BL)
        for k in range(K):
            g = core * K + k
            seg, grp = divmod(g, N_G)
            w = SEG_WARM[seg]
            s0 = SEG_START[seg]
            keep = a[:, w:, k, :]                               # [4, L-w, BL]
            out[grp * BL : (grp + 1) * BL, s0 : s0 + L - w, 0:R] = (
                (ALPHA * SCALE) * keep.transpose(2, 1, 0)
            )
    out[:, :, R:] = ALPHA * Ce
    return out


def kernel(x, m, n, I_mat, _trace=False, **_build_kw):
    global LAST_RESULTS
    from concourse.bass_utils import run_bass_kernel_spmd

    nc = _get_program(**_build_kw)
    in_maps, Ce = _host_prep(
        x, m, n, I_mat,
        q_p=_build_kw.get("q_p", Q_P), q_r=_build_kw.get("q_r", Q_R),
        q_s=_build_kw.get("q_s", Q_S),
    )
    res = run_bass_kernel_spmd(
        nc, in_maps, core_ids=list(range(N_CORES)), trace=_trace
    )
    LAST_RESULTS = res
    return _assemble(res.results, Ce)
